# revision 30
# baseline (speedup 1.0000x reference)
"""Trainium2 Bass kernel for nn_BertOutput (binary-quantized BERT output layer).

Computation (see reference):
    w_scale = mean(|W|, axis=1)                  # [H, 1]
    W_q     = w_scale * sign(W)                  # [H, I]
    x_q     = clip * sign(x / clip)              # [B, S, I]
    h       = x_q @ W_q.T + b                    # [B, S, H]
    out     = LayerNorm(h + input_tensor) * gamma + beta

Structural facts exploited:
  * The matmul operands are exactly +-1: representable exactly in fp8e4m3,
    and the K=4096 accumulation of +-1 terms is exact in fp32 PSUM.  The
    per-output-channel scale (|clip| * mean|W|) is applied after the matmul.
  * fp8 enables MatmulPerfMode.DoubleRow: one instruction contracts TWO
    128-deep k-subtiles (157 TF/s peak), halving tensor-engine time vs bf16.
  * Sign bits survive the fp32->bf16 cast done during the DMA load.  x signs
    are packed PAIRWISE into u16 words -- fp8 sign of x[t, c] in the low
    byte, fp8 sign of x[t, 2048 + c] in the high byte -- with 3 contiguous
    DVE bitwise ops.  One 2-byte transpose then moves BOTH fp8 k-planes at
    once, and the byte-interleaved result is exactly what LDWEIGHTS perf
    mode DoubleRowSwInterleave consumes natively.  SwInterleave loads the
    first element to the largest PE column (reversing token order), which is
    cancelled by assigning tokens to SBUF partitions in reverse order when
    the shard is prepared on the host (a pure row permutation).
  * W is fed TRANSPOSED from the host (a pure layout/sharding choice), so
    it lands k-major and needs no on-device transpose.  It streams on the
    SWDGE ring in PAIR-GROUP order -- k-tile group g together with group
    g+4 -- because DoubleRow block b consumes k-tiles {b, b+16}: blocks
    4g..4g+3 become computable as soon as groups (g, g+4) are signed, while
    the rest of W is still in flight.
  * The matmul work is split: a PSUM-resident chunk (m-tiles 0-1) consumes
    the W pair-groups incrementally during the W stream (the accumulation
    order over k is free), and the remaining m-tiles run back-to-back once
    W is resident.  This removes the serial W-prep phase that previously
    idled the PE for the whole first half of the kernel.
  * The per-channel scale numerator sum_k |W[h,k]| is a cross-partition
    reduction in the W^T layout, computed as ones.T @ |W^T| on the PE;
    |w| tiles are pre-summed in pairs on the DVE to halve the PE matmuls.
  * DMA-xbar transposes lock ALL DMA engines for their whole duration (they
    cannot overlap the HBM loads), so the x tile transposes run on the PE
    array instead (is_transpose matmul; the packed u16 words are bf16
    normals, so a bf16 PE transpose is bit-exact, HW-verified).
  * Only the gpsimd ring can cast f32->bf16 in flight, and concurrent bulk
    on ring+sync queues CONTENDS (~339 GB/s aggregate vs ~390 single), so
    all bulk loads ride the ring in priority order and only the small res /
    output / broadcast traffic uses the sync queue.

Sharding: plain data-parallel over tokens -- 8192 tokens -> 1024 per core.
Each core computes a full LayerNorm over hidden=1024, so no collectives
(measured: the emulated 8-core AllGather costs ~50-60 us end-to-end due to
rank skew + mesh handshakes, which puts it on the critical path; sharing W
through it is a net loss).
"""

import sys

sys.path.insert(0, "/opt/trn_rl_repo")

import numpy as np

import concourse.bass as bass  # noqa: F401  (import side effects / registry)
import concourse.tile as tile
from concourse import bacc, bass_utils, mybir

F32 = mybir.dt.float32
BF16 = mybir.dt.bfloat16
FP8 = mybir.dt.float8e4
U16 = mybir.dt.uint16

HIDDEN = 1024
INTER = 4096
TOKENS = 8192
N_CORES = 8
TPC = TOKENS // N_CORES          # tokens per core = 1024
M_TILES = TPC // 128             # 8 token tiles per core
K_TILES = INTER // 128           # 32 k-tiles of W^T
W_GROUPS = 8                     # W streams as 8 groups of 4 k-tiles (2MB)
NBLK = INTER // 256              # 16 double-k-blocks (DoubleRow: 256 k each)
HALF = INTER // 2                # 2048: pack pairs (k, k + HALF)
A_TILES = 3                      # m-tiles accumulated during the W stream
EPS = 1e-12

TRACE = False                    # set True from test harness to profile
TRACE_ALL_CORES = False

_cache: dict = {}


def _install_ntff_hook():
    """The agent image's antenv package lacks axon_hooks, which silently
    disables NTFF profiling under axon.  Recreate it and wire the ctypes
    hook from trn_agent_boot (profiling/tooling only; the compute path
    does not depend on this)."""
    import types

    import antenv
    if getattr(antenv, "axon_hooks", None) is not None:
        return
    from trn_agent_boot.trn_boot import _ntff_profile_via_ctypes
    mod = types.ModuleType("antenv.axon_hooks")
    mod._hook = _ntff_profile_via_ctypes("/opt/axon/libaxon_pjrt.so")
    mod.get_axon_ntff_profile_hook = lambda: mod._hook

    def _set(h):
        mod._hook = h
    mod.set_axon_ntff_profile_hook = _set
    sys.modules["antenv.axon_hooks"] = mod
    antenv.axon_hooks = mod


def _prepare_x(x_shard: np.ndarray) -> np.ndarray:
    """Sharding-time row permutation: within each 128-token tile, tokens are
    assigned to SBUF partitions in REVERSE order, cancelling SwInterleave's
    first-element-to-largest-column reversal so psum rows come out natural."""
    t = x_shard.reshape(M_TILES, 128, INTER)
    return np.ascontiguousarray(t[:, ::-1, :]).reshape(TPC, INTER)


def _emit_pack(nc, pool, src, dst, tag):
    """Pack sign bits of a bf16 [128, 4096] tile into u16 fp8-sign pairs.

    dst u16 [128, 2048]: word c = lo byte fp8sign(src[:, c]),
                                  hi byte fp8sign(src[:, HALF + c]).
    fp8e4m3 +-1.0 is 0x38 / 0xB8, so:
        lo = (bf16_bits >> 8) & 0x0080  OR'd with  0x0038-from-tsB's 0x3838
        hi = (bf16_bits & 0x8000) | 0x3800
    """
    AT = mybir.AluOpType
    a = src[:, 0:HALF].bitcast(U16)
    b = src[:, HALF:INTER].bitcast(U16)
    tsA = pool.tile([128, HALF], U16, tag=f"{tag}A", bufs=1)
    nc.vector.tensor_scalar(out=tsA[:], in0=a, scalar1=8, scalar2=0x0080,
                            op0=AT.logical_shift_right, op1=AT.bitwise_and)
    tsB = pool.tile([128, HALF], U16, tag=f"{tag}B", bufs=1)
    nc.vector.tensor_scalar(out=tsB[:], in0=b, scalar1=0x8000, scalar2=0x3838,
                            op0=AT.bitwise_and, op1=AT.bitwise_or)
    nc.vector.tensor_tensor(out=dst[:], in0=tsA[:], in1=tsB[:],
                            op=AT.bitwise_or)


def _emit_program(nc, x_ap, res_ap, wt_ap, y_ap, b_ap, g_ap, be_ap,
                  scale_mul: float, use_b: bool, use_gamma: bool,
                  use_beta: bool):
    """Emit the per-core Tile program given DRAM APs.

    wt_ap is W TRANSPOSED ([INTER, HIDDEN]) -- a host-side layout choice so
    the weight lands k-major and needs no on-device transpose.
    """
    AT = mybir.AluOpType
    AF = mybir.ActivationFunctionType
    DRSI = mybir.MatmulPerfMode.DoubleRowSwInterleave
    from concourse.masks import make_identity

    with tile.TileContext(nc) as tc:
        with (
            tc.tile_pool(name="wt", bufs=1) as wt_pool,
            tc.tile_pool(name="wstage", bufs=6) as wstage_pool,
            tc.tile_pool(name="wpk", bufs=2) as wpk_pool,
            tc.tile_pool(name="const", bufs=1) as const_pool,
            tc.tile_pool(name="xio", bufs=2) as xio_pool,
            tc.tile_pool(name="xpk", bufs=2) as xpk_pool,
            tc.tile_pool(name="xt", bufs=3) as xt_pool,
            tc.tile_pool(name="res", bufs=8) as res_pool,
            tc.tile_pool(name="epi", bufs=2) as epi_pool,
            tc.tile_pool(name="stats", bufs=2) as stats_pool,
            tc.tile_pool(name="psum", bufs=6, space="PSUM") as psum_pool,
            tc.tile_pool(name="pst", bufs=2, space="PSUM") as pst_pool,
            tc.tile_pool(name="dram", bufs=1, space="DRAM") as dram_pool,
        ):
            epsT = const_pool.tile([128, 1], F32, tag="epsT")
            nc.vector.memset(epsT[:], float(EPS))
            ones1 = const_pool.tile([128, 1], F32, tag="ones1")
            nc.vector.memset(ones1[:], 1.0)
            ident = const_pool.tile([128, 128], BF16, tag="ident")
            make_identity(nc, ident[:])
            # SBUF f32 accumulator for sum_k |W[h,k]| (replaces a PSUM
            # long-accumulation, freeing 2 PSUM banks for matmul units)
            acc = const_pool.tile([128, HIDDEN], F32, tag="acc")
            nc.vector.memset(acc[:], 0.0)

            # ---------------- DMA dispatch (gpsimd SWDGE ring) ----------------
            # The ring drains strictly in dispatch order, so it doubles as a
            # priority list: x0 first (feeds the transpose front), then the W
            # pair-groups (g, g+4) with x1/x2 interleaved, then the rest of x.
            xins, inps, wgs = {}, {}, {}

            def dispatch_x(m):
                xin = xio_pool.tile([128, INTER], BF16, tag="xin")
                nc.gpsimd.dma_start(xin[:], x_ap[m * 128:(m + 1) * 128, :])
                xins[m] = xin

            def dispatch_w(g):
                wg = wstage_pool.tile([128, 4, HIDDEN], BF16, tag="wld")
                nc.gpsimd.dma_start(
                    wg[:],
                    wt_ap[g * 512:(g + 1) * 512, :].rearrange(
                        "(c p) h -> p c h", p=128))
                wgs[g] = wg

            dispatch_x(0)
            dispatch_w(0)
            dispatch_w(4)
            dispatch_x(1)
            dispatch_w(1)
            dispatch_w(5)
            dispatch_x(2)
            for g in (2, 6, 3, 7):
                dispatch_w(g)
            for m in range(3, M_TILES):
                dispatch_x(m)

            # res tiles ride the HWDGE (sync) queue so they don't dilute the
            # ring's W stream -- but the scheduler dispatches any READY DMA
            # immediately, so without a floor they'd still contend with W
            # for HBM bandwidth.  The wait stamp floats them past the W
            # phase (~70us) while landing before the first epilogue.  All 8
            # are live (bufs=8), so none ever waits on an epilogue.
            with tc.tile_wait_until(0.058):
                for m in range(M_TILES):
                    inp = res_pool.tile([128, HIDDEN], F32, tag="inp")
                    nc.sync.dma_start(inp[:],
                                      res_ap[m * 128:(m + 1) * 128, :])
                    inps[m] = inp

            # ---------------- x front / matmul / epilogue emitters ----------------
            x_fronts = {}

            def emit_x_front(m):
                xin = xins[m]
                xpackU = xpk_pool.tile([128, HALF], U16, tag="xpackU")
                _emit_pack(nc, xpk_pool, xin, xpackU, "xts")
                # transpose the 16 packed blocks on the PE (bit-exact for
                # the 4 sign-pair bf16 normals), staging through PSUM
                xTp = xt_pool.tile([128, NBLK, 128], U16, tag="xTp")
                for grp in range(2):
                    pst = pst_pool.tile([128, 8, 128], BF16, tag="pst")
                    for j in range(8):
                        blk = grp * 8 + j
                        nc.tensor.transpose(
                            pst[:, j, :],
                            xpackU[:, blk * 128:(blk + 1) * 128].bitcast(BF16),
                            ident[:])
                    nc.scalar.copy(
                        xTp[:, grp * 8:(grp + 1) * 8, :].bitcast(BF16),
                        pst[:])
                x_fronts[m] = xTp

            def emit_block_mms(ps2, xTp, b, start, stop):
                # forward interleaved byte-pairs; SwInterleave's column
                # reversal is cancelled by the host-side row reversal.
                # ps2 = (lo, hi) pair of [128, 512] PSUM units (1 bank each)
                lhsT = xTp[:, b, :].bitcast(FP8)
                for n in range(2):
                    nc.tensor.matmul(
                        ps2[n][:],
                        lhsT=lhsT,
                        rhs=wT8[:, b::NBLK, n * 512:(n + 1) * 512],
                        start=start, stop=stop,
                        perf_mode=DRSI)

            def alloc_ps2():
                return tuple(
                    psum_pool.tile([128, 512], F32, tag="psum", name="ps")
                    for _ in range(2))

            def emit_x_mms(m):
                xTp = x_fronts.pop(m)
                ps2 = alloc_ps2()
                for b in range(NBLK):
                    emit_block_mms(ps2, xTp, b, b == 0, b == NBLK - 1)
                return ps2

            def emit_epilogue(m, src2):
                # epilogue: r = src * scaleF + inp (+ bB), then LayerNorm.
                # src2 = (lo, hi) half-APs: PSUM units, or SBUF-copy slices.
                inp = inps[m]
                t = epi_pool.tile([128, HIDDEN], F32, tag="t")
                for n in range(2):
                    nc.vector.tensor_mul(t[:, n * 512:(n + 1) * 512],
                                         src2[n],
                                         scaleF[:, n * 512:(n + 1) * 512])
                # in-place accumulate of the residual (saves an 8KB epi tag)
                nc.vector.tensor_add(t[:], t[:], inp[:])
                r = t
                if use_b:
                    r2 = epi_pool.tile([128, HIDDEN], F32, tag="r2")
                    nc.vector.tensor_add(r2[:], r[:], bB[:])
                    r = r2

                bn6 = stats_pool.tile([128, 2, 6], F32, tag="bn6")
                nc.vector.bn_stats(bn6[:, 0, :], r[:, 0:512])
                nc.vector.bn_stats(bn6[:, 1, :], r[:, 512:1024])
                mv = stats_pool.tile([128, 2], F32, tag="mv")
                nc.vector.bn_aggr(mv[:], bn6[:])
                sd = stats_pool.tile([128, 1], F32, tag="sd")
                nc.scalar.activation(sd[:], mv[:, 1:2], AF.Sqrt,
                                     bias=epsT[:, 0:1])
                rstd = stats_pool.tile([128, 1], F32, tag="rstd")
                nc.vector.reciprocal(rstd[:], sd[:])
                nm = stats_pool.tile([128, 1], F32, tag="nm")
                nc.vector.tensor_scalar(out=nm[:], in0=mv[:, 0:1],
                                        scalar1=rstd[:, 0:1], scalar2=-1.0,
                                        op0=AT.mult, op1=AT.mult)
                y = epi_pool.tile([128, HIDDEN], F32, tag="y")
                nc.scalar.activation(y[:], r[:], AF.Identity,
                                     bias=nm[:, 0:1], scale=rstd[:, 0:1])
                if use_gamma:
                    y2 = epi_pool.tile([128, HIDDEN], F32, tag="y2")
                    nc.vector.tensor_mul(y2[:], y[:], gB[:])
                    y = y2
                if use_beta:
                    y3 = epi_pool.tile([128, HIDDEN], F32, tag="y3")
                    nc.vector.tensor_add(y3[:], y[:], beB[:])
                    y = y3

                nc.sync.dma_start(y_ap[m * 128:(m + 1) * 128, :], y[:])

            # ---------------- W prep + chunk A (during the W stream) -------
            # wT8 fp8 [128, 32, 1024]: (p, kt, h) = fp8 sign W[h, kt*128+p].
            # DoubleRow rhs block b, half n = [:, b::16, n*512:(n+1)*512]
            # (k-pair (b, b+16) matches the x pack pairing (c, 2048+c)).
            wT8 = wt_pool.tile([128, K_TILES, HIDDEN], FP8, tag="wT8",
                               name="wT8")

            emit_x_front(0)
            psumA = [alloc_ps2() for _ in range(A_TILES)]

            for gp in range(4):
                for c in range(4):
                    # sign both halves of the k-pair; alternate engines so
                    # the scalar and vector queues split the work
                    for i, g in enumerate((gp, gp + 4)):
                        kt = g * 4 + c
                        wld = wgs[g]
                        if (c + i) % 2 == 1:
                            wsg = wpk_pool.tile([128, HIDDEN], BF16,
                                                tag="wsg", bufs=1)
                            nc.vector.tensor_scalar(
                                out=wsg[:].bitcast(U16),
                                in0=wld[:, c, :].bitcast(U16),
                                scalar1=0x8000, scalar2=0x3F80,
                                op0=AT.bitwise_and, op1=AT.bitwise_or)
                            nc.vector.tensor_copy(wT8[:, kt, :], wsg[:])
                        else:
                            nc.scalar.sign(wT8[:, kt, :], wld[:, c, :])
                    # |w| of both halves via sign-bit mask, pair-sum on the
                    # DVE, then ones.T @ (|w_lo|+|w_hi|) accumulates the
                    # per-channel scale numerator on the PE
                    wabs = []
                    for g in (gp, gp + 4):
                        wa = wpk_pool.tile([128, HIDDEN], BF16, tag="wabs")
                        nc.vector.tensor_scalar(
                            out=wa[:].bitcast(U16),
                            in0=wgs[g][:, c, :].bitcast(U16),
                            scalar1=0x7FFF, scalar2=None,
                            op0=AT.bitwise_and)
                        wabs.append(wa)
                    wps = wpk_pool.tile([128, HIDDEN], BF16, tag="wps",
                                        bufs=1)
                    nc.vector.tensor_add(wps[:], wabs[0][:], wabs[1][:])
                    nc.vector.tensor_add(acc[:], acc[:], wps[:])
                # chunk A: m-tile t trails t pair-groups behind m0 (its
                # front is only emitted during group t-1), catching up on
                # its missed blocks after the loop.
                for t in range(A_TILES):
                    g2 = gp - t
                    if g2 >= 0:
                        for b in range(g2 * 4, g2 * 4 + 4):
                            emit_block_mms(psumA[t], x_fronts[t], b,
                                           b == 0, b == NBLK - 1 and t == 0)
                if gp == 0:
                    emit_x_front(1)
                if gp == 1:
                    emit_x_front(2)
            for t in range(1, A_TILES):
                for b in range((4 - t) * 4, NBLK):
                    emit_block_mms(psumA[t], x_fronts[t], b,
                                   False, b == NBLK - 1)

            # ---------------- per-channel scale + broadcasts ----------------
            # Cross-partition colsum of acc: one f32 matmul per half, into a
            # [1, 512] f32 view of a borrowed transpose-staging PSUM bank.
            srow = const_pool.tile([1, HIDDEN], F32, tag="srow")
            for n in range(2):
                csp = pst_pool.tile([128, 8, 128], BF16, tag="pst",
                                    name=f"cs{n}")
                csv = csp[0:1, :, :].rearrange("p a b -> p (a b)").bitcast(F32)
                nc.tensor.matmul(csv, lhsT=ones1[:],
                                 rhs=acc[:, n * 512:(n + 1) * 512],
                                 start=True, stop=True)
                nc.scalar.activation(srow[:, n * 512:(n + 1) * 512], csv,
                                     AF.Copy, scale=float(scale_mul))
            scratch = dram_pool.tile([HIDDEN], F32)
            nc.sync.dma_start(
                out=scratch[:].rearrange("(a n) -> a n", a=1), in_=srow[:])
            scaleF = const_pool.tile([128, HIDDEN], F32, tag="scaleF")
            nc.sync.dma_start(
                scaleF[:],
                scratch[:].rearrange("(a n) -> a n", a=1).broadcast_to([128, HIDDEN]))

            bB = gB = beB = None
            if use_b:
                bB = const_pool.tile([128, HIDDEN], F32, tag="bB")
                nc.sync.dma_start(
                    bB[:],
                    b_ap.rearrange("(a n) -> a n", a=1).broadcast_to([128, HIDDEN]))
            if use_gamma:
                gB = const_pool.tile([128, HIDDEN], F32, tag="gB")
                nc.sync.dma_start(
                    gB[:],
                    g_ap.rearrange("(a n) -> a n", a=1).broadcast_to([128, HIDDEN]))
            if use_beta:
                beB = const_pool.tile([128, HIDDEN], F32, tag="beB")
                nc.sync.dma_start(
                    beB[:],
                    be_ap.rearrange("(a n) -> a n", a=1).broadcast_to([128, HIDDEN]))

            # Copy chunk A psums to SBUF immediately: frees their PSUM banks
            # for the tail loop, and breaks the scaleF <-> psum-slot cycle
            # (the epilogue can then wait for scaleF without holding PSUM).
            psA_sb = []
            for m in range(A_TILES):
                sb = epi_pool.tile([128, HIDDEN], F32, tag="psb",
                                   bufs=A_TILES)
                for n in range(2):
                    nc.vector.tensor_copy(sb[:, n * 512:(n + 1) * 512],
                                          psumA[m][n][:])
                psA_sb.append(sb)
                x_fronts.pop(m)

            emit_x_front(3)
            emit_x_front(4)
            for m in range(A_TILES):
                emit_epilogue(m, (psA_sb[m][:, 0:512],
                                  psA_sb[m][:, 512:1024]))

            # ---------------- tail loop over remaining m-tiles ----------------
            # Software-pipelined exactly like the baseline: tile m+2's
            # pack/transpose and tile m-1's epilogue are emitted around tile
            # m's matmuls so the in-order engine queues never stall.
            prev = None
            prev_m = None
            for m in range(A_TILES, M_TILES):
                ps2 = emit_x_mms(m)
                if m + 2 < M_TILES:
                    emit_x_front(m + 2)
                if prev is not None:
                    emit_epilogue(prev_m, (prev[0][:], prev[1][:]))
                prev, prev_m = ps2, m
            emit_epilogue(prev_m, (prev[0][:], prev[1][:]))


def _build(scale_mul: float, use_b: bool, use_gamma: bool, use_beta: bool):
    """Build the SPMD program (identical on all 8 cores).

    scale_mul = |clip_val| / INTER, folded into the per-channel scale.
    """
    nc = bacc.Bacc("TRN2", target_bir_lowering=False, debug=False,
                   num_devices=N_CORES)

    x_ap = nc.dram_tensor("x", [TPC, INTER], F32, kind="ExternalInput").ap()
    res_ap = nc.dram_tensor("res", [TPC, HIDDEN], F32, kind="ExternalInput").ap()
    wt_ap = nc.dram_tensor("wt", [INTER, HIDDEN], F32, kind="ExternalInput").ap()
    b_ap = g_ap = be_ap = None
    if use_b:
        b_ap = nc.dram_tensor("bvec", [HIDDEN], F32, kind="ExternalInput").ap()
    if use_gamma:
        g_ap = nc.dram_tensor("gvec", [HIDDEN], F32, kind="ExternalInput").ap()
    if use_beta:
        be_ap = nc.dram_tensor("bevec", [HIDDEN], F32, kind="ExternalInput").ap()
    y_ap = nc.dram_tensor("y", [TPC, HIDDEN], F32, kind="ExternalOutput").ap()

    _emit_program(nc, x_ap, res_ap, wt_ap, y_ap, b_ap, g_ap, be_ap,
                  scale_mul, use_b, use_gamma, use_beta)
    nc.compile()
    return nc


_last_results = None


def kernel(hidden_states, input_tensor, W, b, clip_val, gamma, beta):
    global _last_results
    hidden_states = np.asarray(hidden_states)
    input_tensor = np.asarray(input_tensor)
    W = np.asarray(W, dtype=np.float32)
    b = np.asarray(b, dtype=np.float32)
    gamma = np.asarray(gamma, dtype=np.float32)
    beta = np.asarray(beta, dtype=np.float32)
    clip = float(np.asarray(clip_val))

    use_b = bool(np.any(b != 0.0))
    use_gamma = bool(np.any(gamma != 1.0))
    use_beta = bool(np.any(beta != 0.0))
    scale_mul = abs(clip) / INTER

    key = (scale_mul, use_b, use_gamma, use_beta)
    if key not in _cache:
        _cache[key] = _build(scale_mul, use_b, use_gamma, use_beta)
    nc = _cache[key]

    hs = np.ascontiguousarray(
        hidden_states.reshape(TOKENS, INTER).astype(np.float32, copy=False))
    rs = np.ascontiguousarray(
        input_tensor.reshape(TOKENS, HIDDEN).astype(np.float32, copy=False))
    Wc = np.ascontiguousarray(W.T)   # layout choice: weight fed k-major

    in_maps = []
    for c in range(N_CORES):
        m = {
            "x": _prepare_x(hs[c * TPC:(c + 1) * TPC]),
            "res": np.ascontiguousarray(rs[c * TPC:(c + 1) * TPC]),
            "wt": Wc,
        }
        if use_b:
            m["bvec"] = b
        if use_gamma:
            m["gvec"] = gamma
        if use_beta:
            m["bevec"] = beta
        in_maps.append(m)

    kwargs = {}
    if TRACE:
        _install_ntff_hook()
        kwargs["trace"] = True
        if TRACE_ALL_CORES:
            kwargs["trace_cores"] = list(range(N_CORES))
    res = bass_utils.run_bass_kernel_spmd(
        nc, in_maps, core_ids=list(range(N_CORES)), **kwargs)
    _last_results = res

    y = np.concatenate([res.results[c]["y"] for c in range(N_CORES)], axis=0)
    return y.reshape(hidden_states.shape[:-1] + (HIDDEN,)).astype(np.float32)


# revision 35
# speedup vs baseline: 1.0899x; 1.0899x over previous
"""Trainium2 Bass kernel for nn_BertOutput (binary-quantized BERT output layer).

Computation (see reference):
    w_scale = mean(|W|, axis=1)                  # [H, 1]
    W_q     = w_scale * sign(W)                  # [H, I]
    x_q     = clip * sign(x / clip)              # [B, S, I]
    h       = x_q @ W_q.T + b                    # [B, S, H]
    out     = LayerNorm(h + input_tensor) * gamma + beta

Structural facts exploited:
  * The matmul operands are exactly +-1: representable exactly in fp8e4m3,
    and the K=4096 accumulation of +-1 terms is exact in fp32 PSUM.  The
    per-output-channel scale (|clip| * mean|W|) is applied after the matmul.
  * fp8 enables MatmulPerfMode.DoubleRow: one instruction contracts TWO
    128-deep k-subtiles (157 TF/s peak), halving tensor-engine time vs bf16.
  * Sign bits survive the fp32->bf16 cast done during the DMA load.  x signs
    are packed PAIRWISE into u16 words -- fp8 sign of x[t, c] in the low
    byte, fp8 sign of x[t, 2048 + c] in the high byte -- with 3 contiguous
    DVE bitwise ops.  One 2-byte transpose then moves BOTH fp8 k-planes at
    once, and the byte-interleaved result is exactly what LDWEIGHTS perf
    mode DoubleRowSwInterleave consumes natively.  SwInterleave loads the
    first element to the largest PE column (reversing token order), which is
    cancelled by assigning tokens to SBUF partitions in reverse order when
    the shard is prepared on the host (a pure row permutation).
  * W is fed TRANSPOSED from the host (a pure layout/sharding choice), so
    it lands k-major and needs no on-device transpose.  It streams on the
    SWDGE ring in PAIR-GROUP order -- k-tile group g together with group
    g+4 -- because DoubleRow block b consumes k-tiles {b, b+16}: blocks
    4g..4g+3 become computable as soon as groups (g, g+4) are signed, while
    the rest of W is still in flight.
  * The matmul work is split: a PSUM-resident chunk (m-tiles 0-1) consumes
    the W pair-groups incrementally during the W stream (the accumulation
    order over k is free), and the remaining m-tiles run back-to-back once
    W is resident.  This removes the serial W-prep phase that previously
    idled the PE for the whole first half of the kernel.
  * The per-channel scale numerator sum_k |W[h,k]| is a cross-partition
    reduction in the W^T layout, computed as ones.T @ |W^T| on the PE;
    |w| tiles are pre-summed in pairs on the DVE to halve the PE matmuls.
  * DMA-xbar transposes lock ALL DMA engines for their whole duration (they
    cannot overlap the HBM loads), so the x tile transposes run on the PE
    array instead (is_transpose matmul; the packed u16 words are bf16
    normals, so a bf16 PE transpose is bit-exact, HW-verified).
  * Only the gpsimd ring can cast f32->bf16 in flight, and concurrent bulk
    on ring+sync queues CONTENDS (~339 GB/s aggregate vs ~390 single), so
    all bulk loads ride the ring in priority order and only the small res /
    output / broadcast traffic uses the sync queue.

Sharding: plain data-parallel over tokens -- 8192 tokens -> 1024 per core.
Each core computes a full LayerNorm over hidden=1024, so no collectives
(measured: the emulated 8-core AllGather costs ~50-60 us end-to-end due to
rank skew + mesh handshakes, which puts it on the critical path; sharing W
through it is a net loss).
"""

import sys

sys.path.insert(0, "/opt/trn_rl_repo")

import numpy as np

import concourse.bass as bass  # noqa: F401  (import side effects / registry)
import concourse.tile as tile
from concourse import bacc, bass_utils, mybir

F32 = mybir.dt.float32
BF16 = mybir.dt.bfloat16
FP8 = mybir.dt.float8e4
U16 = mybir.dt.uint16

HIDDEN = 1024
INTER = 4096
TOKENS = 8192
N_CORES = 8
TPC = TOKENS // N_CORES          # tokens per core = 1024
M_TILES = TPC // 128             # 8 token tiles per core
K_TILES = INTER // 128           # 32 k-tiles of W^T
W_GROUPS = 8                     # W streams as 8 groups of 4 k-tiles (2MB)
NBLK = INTER // 256              # 16 double-k-blocks (DoubleRow: 256 k each)
HALF = INTER // 2                # 2048: pack pairs (k, k + HALF)
A_TILES = 1                      # m-tiles accumulated during the W stream
EPS = 1e-12

TRACE = False                    # set True from test harness to profile
TRACE_ALL_CORES = False

_cache: dict = {}


def _install_ntff_hook():
    """The agent image's antenv package lacks axon_hooks, which silently
    disables NTFF profiling under axon.  Recreate it and wire the ctypes
    hook from trn_agent_boot (profiling/tooling only; the compute path
    does not depend on this)."""
    import types

    import antenv
    if getattr(antenv, "axon_hooks", None) is not None:
        return
    from trn_agent_boot.trn_boot import _ntff_profile_via_ctypes
    mod = types.ModuleType("antenv.axon_hooks")
    mod._hook = _ntff_profile_via_ctypes("/opt/axon/libaxon_pjrt.so")
    mod.get_axon_ntff_profile_hook = lambda: mod._hook

    def _set(h):
        mod._hook = h
    mod.set_axon_ntff_profile_hook = _set
    sys.modules["antenv.axon_hooks"] = mod
    antenv.axon_hooks = mod


def _prepare_x(x_shard: np.ndarray) -> np.ndarray:
    """Sharding-time row permutation: within each 128-token tile, tokens are
    assigned to SBUF partitions in REVERSE order, cancelling SwInterleave's
    first-element-to-largest-column reversal so psum rows come out natural."""
    t = x_shard.reshape(M_TILES, 128, INTER)
    return np.ascontiguousarray(t[:, ::-1, :]).reshape(TPC, INTER)


def _emit_pack(nc, pool, src, dst, tag):
    """Pack sign bits of a bf16 [128, 4096] tile into u16 fp8-sign pairs.

    dst u16 [128, 2048]: word c = lo byte fp8sign(src[:, c]),
                                  hi byte fp8sign(src[:, HALF + c]).
    fp8e4m3 +-1.0 is 0x38 / 0xB8, so:
        lo = (bf16_bits >> 8) & 0x0080  OR'd with  0x0038-from-tsB's 0x3838
        hi = (bf16_bits & 0x8000) | 0x3800
    """
    AT = mybir.AluOpType
    a = src[:, 0:HALF].bitcast(U16)
    b = src[:, HALF:INTER].bitcast(U16)
    tsA = pool.tile([128, HALF], U16, tag=f"{tag}A", bufs=1)
    nc.vector.tensor_scalar(out=tsA[:], in0=a, scalar1=8, scalar2=0x0080,
                            op0=AT.logical_shift_right, op1=AT.bitwise_and)
    tsB = pool.tile([128, HALF], U16, tag=f"{tag}B", bufs=1)
    nc.vector.tensor_scalar(out=tsB[:], in0=b, scalar1=0x8000, scalar2=0x3838,
                            op0=AT.bitwise_and, op1=AT.bitwise_or)
    nc.vector.tensor_tensor(out=dst[:], in0=tsA[:], in1=tsB[:],
                            op=AT.bitwise_or)


def _emit_program(nc, x_ap, res_ap, wt_ap, y_ap, b_ap, g_ap, be_ap,
                  scale_mul: float, use_b: bool, use_gamma: bool,
                  use_beta: bool):
    """Emit the per-core Tile program given DRAM APs.

    wt_ap is W TRANSPOSED ([INTER, HIDDEN]) -- a host-side layout choice so
    the weight lands k-major and needs no on-device transpose.
    """
    AT = mybir.AluOpType
    AF = mybir.ActivationFunctionType
    DRSI = mybir.MatmulPerfMode.DoubleRowSwInterleave
    from concourse.masks import make_identity

    with tile.TileContext(nc) as tc:
        with (
            tc.tile_pool(name="wt", bufs=1) as wt_pool,
            tc.tile_pool(name="wstage", bufs=5) as wstage_pool,
            tc.tile_pool(name="wpk", bufs=2) as wpk_pool,
            tc.tile_pool(name="const", bufs=1) as const_pool,
            tc.tile_pool(name="xio", bufs=3) as xio_pool,
            tc.tile_pool(name="xpk", bufs=2) as xpk_pool,
            tc.tile_pool(name="xt", bufs=3) as xt_pool,
            tc.tile_pool(name="res", bufs=8) as res_pool,
            tc.tile_pool(name="epi", bufs=2) as epi_pool,
            tc.tile_pool(name="stats", bufs=2) as stats_pool,
            tc.tile_pool(name="psum", bufs=6, space="PSUM") as psum_pool,
            tc.tile_pool(name="pst", bufs=2, space="PSUM") as pst_pool,
            tc.tile_pool(name="dram", bufs=1, space="DRAM") as dram_pool,
        ):
            epsT = const_pool.tile([128, 1], F32, tag="epsT")
            nc.vector.memset(epsT[:], float(EPS))
            ones1 = const_pool.tile([128, 1], F32, tag="ones1")
            nc.vector.memset(ones1[:], 1.0)
            ident = const_pool.tile([128, 128], BF16, tag="ident")
            make_identity(nc, ident[:])
            # SBUF f32 accumulator for sum_k |W[h,k]| (replaces a PSUM
            # long-accumulation, freeing 2 PSUM banks for matmul units)
            acc = const_pool.tile([128, HIDDEN], F32, tag="acc")
            nc.vector.memset(acc[:], 0.0)

            # ---------------- DMA dispatch (gpsimd SWDGE ring) ----------------
            # The ring drains strictly in dispatch order, so it doubles as a
            # priority list: x0 first (feeds the transpose front), then the W
            # pair-groups (g, g+4) with x1/x2 interleaved, then the rest of x.
            xins, inps, wgs = {}, {}, {}

            def dispatch_x(m):
                xin = xio_pool.tile([128, INTER], BF16, tag="xin")
                nc.gpsimd.dma_start(xin[:], x_ap[m * 128:(m + 1) * 128, :])
                xins[m] = xin

            def dispatch_w(g):
                wg = wstage_pool.tile([128, 4, HIDDEN], BF16, tag="wld")
                nc.gpsimd.dma_start(
                    wg[:],
                    wt_ap[g * 512:(g + 1) * 512, :].rearrange(
                        "(c p) h -> p c h", p=128))
                wgs[g] = wg

            # Only x0 rides ahead of the W stream (each promoted x tile
            # delays W-done by its transfer time, which roughly cancels the
            # matmul work the chunk-A overlap buys back -- so just one).
            dispatch_x(0)
            for g in (0, 4, 1, 5, 2, 6, 3, 7):
                dispatch_w(g)
            for m in range(1, 4):
                dispatch_x(m)
            # res rides the ring too (a concurrent sync stream would steal
            # HBM bandwidth from the W phase).  All 8 tiles are live
            # (bufs=8) so these DMAs are ready immediately; the scheduler
            # slots them right after the W groups, in time for the first
            # epilogues.
            for m in range(M_TILES):
                inp = res_pool.tile([128, HIDDEN], F32, tag="inp")
                nc.gpsimd.dma_start(inp[:], res_ap[m * 128:(m + 1) * 128, :])
                inps[m] = inp
            for m in range(4, M_TILES):
                dispatch_x(m)

            # ---------------- x front / matmul / epilogue emitters ----------------
            x_fronts = {}

            def emit_x_front(m):
                xin = xins[m]
                xpackU = xpk_pool.tile([128, HALF], U16, tag="xpackU")
                _emit_pack(nc, xpk_pool, xin, xpackU, "xts")
                # transpose the 16 packed blocks on the PE (bit-exact for
                # the 4 sign-pair bf16 normals), staging through PSUM
                xTp = xt_pool.tile([128, NBLK, 128], U16, tag="xTp")
                for grp in range(2):
                    pst = pst_pool.tile([128, 8, 128], BF16, tag="pst")
                    for j in range(8):
                        blk = grp * 8 + j
                        nc.tensor.transpose(
                            pst[:, j, :],
                            xpackU[:, blk * 128:(blk + 1) * 128].bitcast(BF16),
                            ident[:])
                    nc.scalar.copy(
                        xTp[:, grp * 8:(grp + 1) * 8, :].bitcast(BF16),
                        pst[:])
                x_fronts[m] = xTp

            def emit_block_mms(ps2, xTp, b, start, stop):
                # forward interleaved byte-pairs; SwInterleave's column
                # reversal is cancelled by the host-side row reversal.
                # ps2 = (lo, hi) pair of [128, 512] PSUM units (1 bank each)
                lhsT = xTp[:, b, :].bitcast(FP8)
                for n in range(2):
                    nc.tensor.matmul(
                        ps2[n][:],
                        lhsT=lhsT,
                        rhs=wT8[:, b::NBLK, n * 512:(n + 1) * 512],
                        start=start, stop=stop,
                        perf_mode=DRSI)

            def alloc_ps2():
                return tuple(
                    psum_pool.tile([128, 512], F32, tag="psum", name="ps")
                    for _ in range(2))

            def emit_x_mms(m):
                xTp = x_fronts.pop(m)
                ps2 = alloc_ps2()
                for b in range(NBLK):
                    emit_block_mms(ps2, xTp, b, b == 0, b == NBLK - 1)
                return ps2

            def emit_epilogue(m, src2):
                # epilogue: r = src * scaleF + inp (+ bB), then LayerNorm.
                # src2 = (lo, hi) half-APs: PSUM units, or SBUF-copy slices.
                inp = inps[m]
                t = epi_pool.tile([128, HIDDEN], F32, tag="t")
                for n in range(2):
                    nc.vector.tensor_mul(t[:, n * 512:(n + 1) * 512],
                                         src2[n],
                                         scaleF[:, n * 512:(n + 1) * 512])
                # in-place accumulate of the residual (saves an 8KB epi tag)
                nc.vector.tensor_add(t[:], t[:], inp[:])
                r = t
                if use_b:
                    r2 = epi_pool.tile([128, HIDDEN], F32, tag="r2")
                    nc.vector.tensor_add(r2[:], r[:], bB[:])
                    r = r2

                bn6 = stats_pool.tile([128, 2, 6], F32, tag="bn6")
                nc.vector.bn_stats(bn6[:, 0, :], r[:, 0:512])
                nc.vector.bn_stats(bn6[:, 1, :], r[:, 512:1024])
                mv = stats_pool.tile([128, 2], F32, tag="mv")
                nc.vector.bn_aggr(mv[:], bn6[:])
                sd = stats_pool.tile([128, 1], F32, tag="sd")
                nc.scalar.activation(sd[:], mv[:, 1:2], AF.Sqrt,
                                     bias=epsT[:, 0:1])
                rstd = stats_pool.tile([128, 1], F32, tag="rstd")
                nc.vector.reciprocal(rstd[:], sd[:])
                nm = stats_pool.tile([128, 1], F32, tag="nm")
                nc.vector.tensor_scalar(out=nm[:], in0=mv[:, 0:1],
                                        scalar1=rstd[:, 0:1], scalar2=-1.0,
                                        op0=AT.mult, op1=AT.mult)
                y = epi_pool.tile([128, HIDDEN], F32, tag="y")
                nc.scalar.activation(y[:], r[:], AF.Identity,
                                     bias=nm[:, 0:1], scale=rstd[:, 0:1])
                if use_gamma:
                    y2 = epi_pool.tile([128, HIDDEN], F32, tag="y2")
                    nc.vector.tensor_mul(y2[:], y[:], gB[:])
                    y = y2
                if use_beta:
                    y3 = epi_pool.tile([128, HIDDEN], F32, tag="y3")
                    nc.vector.tensor_add(y3[:], y[:], beB[:])
                    y = y3

                nc.sync.dma_start(y_ap[m * 128:(m + 1) * 128, :], y[:])

            # ---------------- W prep + chunk A (during the W stream) -------
            # wT8 fp8 [128, 32, 1024]: (p, kt, h) = fp8 sign W[h, kt*128+p].
            # DoubleRow rhs block b, half n = [:, b::16, n*512:(n+1)*512]
            # (k-pair (b, b+16) matches the x pack pairing (c, 2048+c)).
            wT8 = wt_pool.tile([128, K_TILES, HIDDEN], FP8, tag="wT8",
                               name="wT8")

            emit_x_front(0)
            psumA = [alloc_ps2() for _ in range(A_TILES)]

            for gp in range(4):
                for c in range(4):
                    # sign both halves of the k-pair; alternate engines so
                    # the scalar and vector queues split the work
                    for i, g in enumerate((gp, gp + 4)):
                        kt = g * 4 + c
                        wld = wgs[g]
                        if (c + i) % 2 == 1:
                            wsg = wpk_pool.tile([128, HIDDEN], BF16,
                                                tag="wsg", bufs=1)
                            nc.vector.tensor_scalar(
                                out=wsg[:].bitcast(U16),
                                in0=wld[:, c, :].bitcast(U16),
                                scalar1=0x8000, scalar2=0x3F80,
                                op0=AT.bitwise_and, op1=AT.bitwise_or)
                            nc.vector.tensor_copy(wT8[:, kt, :], wsg[:])
                        else:
                            nc.scalar.sign(wT8[:, kt, :], wld[:, c, :])
                    # |w| of both halves via sign-bit mask, pair-sum on the
                    # DVE, then ones.T @ (|w_lo|+|w_hi|) accumulates the
                    # per-channel scale numerator on the PE
                    wabs = []
                    for g in (gp, gp + 4):
                        wa = wpk_pool.tile([128, HIDDEN], BF16, tag="wabs")
                        nc.vector.tensor_scalar(
                            out=wa[:].bitcast(U16),
                            in0=wgs[g][:, c, :].bitcast(U16),
                            scalar1=0x7FFF, scalar2=None,
                            op0=AT.bitwise_and)
                        wabs.append(wa)
                    wps = wpk_pool.tile([128, HIDDEN], BF16, tag="wps",
                                        bufs=1)
                    nc.vector.tensor_add(wps[:], wabs[0][:], wabs[1][:])
                    nc.vector.tensor_add(acc[:], acc[:], wps[:])
                # chunk A: m-tile t trails t pair-groups behind m0 (its
                # front is only emitted during group t-1), catching up on
                # its missed blocks after the loop.
                for t in range(A_TILES):
                    g2 = gp - t
                    if g2 >= 0:
                        for b in range(g2 * 4, g2 * 4 + 4):
                            emit_block_mms(psumA[t], x_fronts[t], b,
                                           b == 0, b == NBLK - 1 and t == 0)
                if gp == 0:
                    emit_x_front(1)
                if gp == 1:
                    emit_x_front(2)
            for t in range(1, A_TILES):
                for b in range((4 - t) * 4, NBLK):
                    emit_block_mms(psumA[t], x_fronts[t], b,
                                   False, b == NBLK - 1)

            # ---------------- per-channel scale + broadcasts ----------------
            # Cross-partition colsum of acc: one f32 matmul per half, into a
            # [1, 512] f32 view of a borrowed transpose-staging PSUM bank.
            srow = const_pool.tile([1, HIDDEN], F32, tag="srow")
            for n in range(2):
                csp = pst_pool.tile([128, 8, 128], BF16, tag="pst",
                                    name=f"cs{n}")
                csv = csp[0:1, :, :].rearrange("p a b -> p (a b)").bitcast(F32)
                nc.tensor.matmul(csv, lhsT=ones1[:],
                                 rhs=acc[:, n * 512:(n + 1) * 512],
                                 start=True, stop=True)
                nc.scalar.activation(srow[:, n * 512:(n + 1) * 512], csv,
                                     AF.Copy, scale=float(scale_mul))
            scratch = dram_pool.tile([HIDDEN], F32)
            nc.sync.dma_start(
                out=scratch[:].rearrange("(a n) -> a n", a=1), in_=srow[:])
            scaleF = const_pool.tile([128, HIDDEN], F32, tag="scaleF")
            nc.sync.dma_start(
                scaleF[:],
                scratch[:].rearrange("(a n) -> a n", a=1).broadcast_to([128, HIDDEN]))

            bB = gB = beB = None
            if use_b:
                bB = const_pool.tile([128, HIDDEN], F32, tag="bB")
                nc.sync.dma_start(
                    bB[:],
                    b_ap.rearrange("(a n) -> a n", a=1).broadcast_to([128, HIDDEN]))
            if use_gamma:
                gB = const_pool.tile([128, HIDDEN], F32, tag="gB")
                nc.sync.dma_start(
                    gB[:],
                    g_ap.rearrange("(a n) -> a n", a=1).broadcast_to([128, HIDDEN]))
            if use_beta:
                beB = const_pool.tile([128, HIDDEN], F32, tag="beB")
                nc.sync.dma_start(
                    beB[:],
                    be_ap.rearrange("(a n) -> a n", a=1).broadcast_to([128, HIDDEN]))

            # Copy chunk A psums to SBUF immediately: frees their PSUM banks
            # for the tail loop, and breaks the scaleF <-> psum-slot cycle
            # (the epilogue can then wait for scaleF without holding PSUM).
            psA_sb = []
            for m in range(A_TILES):
                sb = epi_pool.tile([128, HIDDEN], F32, tag="psb",
                                   bufs=A_TILES)
                for n in range(2):
                    nc.vector.tensor_copy(sb[:, n * 512:(n + 1) * 512],
                                          psumA[m][n][:])
                psA_sb.append(sb)
                x_fronts.pop(m)

            for m in range(A_TILES):
                emit_epilogue(m, (psA_sb[m][:, 0:512],
                                  psA_sb[m][:, 512:1024]))

            # ---------------- tail loop over remaining m-tiles ----------------
            # Software-pipelined exactly like the baseline: tile m+2's
            # pack/transpose and tile m-1's epilogue are emitted around tile
            # m's matmuls so the in-order engine queues never stall.
            prev = None
            prev_m = None
            for m in range(A_TILES, M_TILES):
                ps2 = emit_x_mms(m)
                if m + 2 < M_TILES:
                    emit_x_front(m + 2)
                if prev is not None:
                    emit_epilogue(prev_m, (prev[0][:], prev[1][:]))
                prev, prev_m = ps2, m
            emit_epilogue(prev_m, (prev[0][:], prev[1][:]))


def _build(scale_mul: float, use_b: bool, use_gamma: bool, use_beta: bool):
    """Build the SPMD program (identical on all 8 cores).

    scale_mul = |clip_val| / INTER, folded into the per-channel scale.
    """
    nc = bacc.Bacc("TRN2", target_bir_lowering=False, debug=False,
                   num_devices=N_CORES)

    x_ap = nc.dram_tensor("x", [TPC, INTER], F32, kind="ExternalInput").ap()
    res_ap = nc.dram_tensor("res", [TPC, HIDDEN], F32, kind="ExternalInput").ap()
    wt_ap = nc.dram_tensor("wt", [INTER, HIDDEN], F32, kind="ExternalInput").ap()
    b_ap = g_ap = be_ap = None
    if use_b:
        b_ap = nc.dram_tensor("bvec", [HIDDEN], F32, kind="ExternalInput").ap()
    if use_gamma:
        g_ap = nc.dram_tensor("gvec", [HIDDEN], F32, kind="ExternalInput").ap()
    if use_beta:
        be_ap = nc.dram_tensor("bevec", [HIDDEN], F32, kind="ExternalInput").ap()
    y_ap = nc.dram_tensor("y", [TPC, HIDDEN], F32, kind="ExternalOutput").ap()

    _emit_program(nc, x_ap, res_ap, wt_ap, y_ap, b_ap, g_ap, be_ap,
                  scale_mul, use_b, use_gamma, use_beta)
    nc.compile()
    return nc


_last_results = None


def kernel(hidden_states, input_tensor, W, b, clip_val, gamma, beta):
    global _last_results
    hidden_states = np.asarray(hidden_states)
    input_tensor = np.asarray(input_tensor)
    W = np.asarray(W, dtype=np.float32)
    b = np.asarray(b, dtype=np.float32)
    gamma = np.asarray(gamma, dtype=np.float32)
    beta = np.asarray(beta, dtype=np.float32)
    clip = float(np.asarray(clip_val))

    use_b = bool(np.any(b != 0.0))
    use_gamma = bool(np.any(gamma != 1.0))
    use_beta = bool(np.any(beta != 0.0))
    scale_mul = abs(clip) / INTER

    key = (scale_mul, use_b, use_gamma, use_beta)
    if key not in _cache:
        _cache[key] = _build(scale_mul, use_b, use_gamma, use_beta)
    nc = _cache[key]

    hs = np.ascontiguousarray(
        hidden_states.reshape(TOKENS, INTER).astype(np.float32, copy=False))
    rs = np.ascontiguousarray(
        input_tensor.reshape(TOKENS, HIDDEN).astype(np.float32, copy=False))
    Wc = np.ascontiguousarray(W.T)   # layout choice: weight fed k-major

    in_maps = []
    for c in range(N_CORES):
        m = {
            "x": _prepare_x(hs[c * TPC:(c + 1) * TPC]),
            "res": np.ascontiguousarray(rs[c * TPC:(c + 1) * TPC]),
            "wt": Wc,
        }
        if use_b:
            m["bvec"] = b
        if use_gamma:
            m["gvec"] = gamma
        if use_beta:
            m["bevec"] = beta
        in_maps.append(m)

    kwargs = {}
    if TRACE:
        _install_ntff_hook()
        kwargs["trace"] = True
        if TRACE_ALL_CORES:
            kwargs["trace_cores"] = list(range(N_CORES))
    res = bass_utils.run_bass_kernel_spmd(
        nc, in_maps, core_ids=list(range(N_CORES)), **kwargs)
    _last_results = res

    y = np.concatenate([res.results[c]["y"] for c in range(N_CORES)], axis=0)
    return y.reshape(hidden_states.shape[:-1] + (HIDDEN,)).astype(np.float32)
